# revision 2
# baseline (speedup 1.0000x reference)
"""Axial (per-row) pair attention kernel for Trainium2, 8-core SPMD.

Contract: kernel(**inputs) takes the FULL unsharded inputs from
setup_inputs() and returns the FULL (2,128,128,256) float32 output.

Sharding: the (b, s1) row axis (2*128 = 256 independent attention rows) is
split evenly across 8 NeuronCores; each core runs the identical Bass program
on its 32-row slice.

v2 changes vs baseline:
 - No per-head q/k repack. q/k are copied full-width (128 partitions) from
   PSUM, and the per-head scores matmuls are replaced by one matmul per
   (chunk, row) with lhsT = the full 128-channel k chunk and rhs = a
   zero-blocked q tile (head g's 32 channels live at partitions 32g, all
   other partitions zero), giving all 4 heads' scoresT in one N=512 matmul.
 - Softmax exp runs as one fused ACT op per (row, chunk-half) from PSUM.
 - LN-apply and the rotary cos/sin multiplies run on the idle GpSimd (Pool)
   engine (SBUF-only ops); PSUM->SBUF copies are balanced across DVE/ACT.
 - Output is DMA'd as fp16 and upcast on the host.
"""

import numpy as np

import concourse.bass as bass
import concourse.mybir as mybir
import concourse.tile as tile
from concourse import bacc
from concourse.bass_utils import run_bass_kernel_spmd
from concourse.masks import make_identity

N_CORES = 8
B, S, D = 2, 128, 256
H, HD, ROT = 8, 32, 32
NROWS = B * S
RPC = NROWS // N_CORES  # rows per core = 32
SCALE = HD ** -0.5
LN_EPS = 1e-5
MASK_BIAS = -1e9

F32 = mybir.dt.float32
F16 = mybir.dt.float16


def _build_bass() -> bass.Bass:
    nc = bacc.Bacc(None)

    x = nc.dram_tensor("x", [RPC, S, D], F16, kind="ExternalInput")
    cos_t = nc.dram_tensor("cos_t", [ROT, RPC, S], F16, kind="ExternalInput")
    sin_t = nc.dram_tensor("sin_t", [ROT, RPC, S], F16, kind="ExternalInput")
    maskb = nc.dram_tensor("maskb", [S, RPC], F32, kind="ExternalInput")
    wqkv = nc.dram_tensor("wqkv", [2, 128, 3 * D], F16, kind="ExternalInput")
    wout = nc.dram_tensor("wout", [2, 128, D], F16, kind="ExternalInput")
    rt = nc.dram_tensor("rt", [ROT, ROT], F16, kind="ExternalInput")
    y = nc.dram_tensor("y", [RPC, S, D], F16, kind="ExternalOutput")

    with tile.TileContext(nc) as tc:
        with (
            tc.tile_pool(name="consts", bufs=1) as consts,
            tc.tile_pool(name="xpool", bufs=RPC) as xpool,
            tc.tile_pool(name="lnpool", bufs=4) as lnpool,
            tc.tile_pool(name="tpool", bufs=3) as tpool,
            tc.tile_pool(name="qkpool", bufs=2) as qkpool,
            tc.tile_pool(name="qzpool", bufs=3) as qzpool,
            tc.tile_pool(name="vpool", bufs=3) as vpool,
            tc.tile_pool(name="epool", bufs=3) as epool,
            tc.tile_pool(name="apool", bufs=3) as apool,
            tc.tile_pool(name="ypool", bufs=3) as ypool,
            tc.tile_pool(name="ps_t", bufs=1, space="PSUM") as ps_t,
            tc.tile_pool(name="ps_qk", bufs=1, space="PSUM") as ps_qk,
            tc.tile_pool(name="ps_rh", bufs=1, space="PSUM") as ps_rh,
            tc.tile_pool(name="ps_s", bufs=2, space="PSUM") as ps_s,
            tc.tile_pool(name="ps_o", bufs=2, space="PSUM") as ps_o,
        ):
            # ---- constants ----
            ident = consts.tile([128, 128], F16)
            make_identity(nc, ident)
            mv_all = consts.tile([S, RPC, 2], F32)
            rstd_all = consts.tile([S, RPC], F32)
            eps_sb = consts.tile([128, 1], F32)
            nc.vector.memset(eps_sb, LN_EPS)
            x_tiles = []

            def load_pair(p):
                x_sb = xpool.tile([S, 2, D], F16)
                nc.sync.dma_start(
                    out=x_sb, in_=x[2 * p:2 * p + 2].rearrange("r t d -> t r d")
                )
                x_tiles.extend([x_sb[:, 0, :], x_sb[:, 1, :]])

            def stats_pair(p):
                stats = lnpool.tile([S, 2, 6], F32, tag="stats")
                for j in range(2):
                    nc.vector.bn_stats(
                        out=stats[:, j, :], in_=x_tiles[2 * p + j]
                    )
                    nc.vector.bn_aggr(
                        out=mv_all[:, 2 * p + j, :], in_=stats[:, j, :]
                    )

            # first 4 x pairs, then constants, then the rest of x in two
            # batched DMAs; LN stats run as the data lands
            for p in range(4):
                load_pair(p)
            wqkv_sb = consts.tile([128, 2, 3 * D], F16)
            for c in range(2):
                nc.sync.dma_start(out=wqkv_sb[:, c, :], in_=wqkv[c])
            rt_sb = consts.tile([ROT, ROT], F16)
            nc.sync.dma_start(out=rt_sb, in_=rt[:])
            maskb_sb = consts.tile([S, RPC], F32)
            nc.sync.dma_start(out=maskb_sb, in_=maskb[:])
            cos_sb = consts.tile([ROT, RPC, S], F16)
            sin_sb = consts.tile([ROT, RPC, S], F16)
            nc.sync.dma_start(out=cos_sb, in_=cos_t[:])
            nc.sync.dma_start(out=sin_sb, in_=sin_t[:])
            wout_sb = consts.tile([128, 2, D], F16)
            for c in range(2):
                nc.sync.dma_start(out=wout_sb[:, c, :], in_=wout[c])
            for p in range(4):
                stats_pair(p)
            xbig = consts.tile([S, 12, 2, D], F16)
            for h in range(2):
                nc.sync.dma_start(
                    out=xbig[:, 6 * h:6 * h + 6, :, :],
                    in_=x[8 + 12 * h:8 + 12 * h + 12].rearrange(
                        "(p r) t d -> t p r d", r=2
                    ),
                )
            for p in range(4, RPC // 2):
                x_tiles.extend(
                    [xbig[:, p - 4, 0, :], xbig[:, p - 4, 1, :]]
                )
            for p in range(4, RPC // 2):
                stats_pair(p)
            # rstd rows 0-7 now (Sqrt table loaded once); rows 8-31 at i==1,
            # still before the first Exp so the act table loads stay at two
            nc.scalar.activation(
                out=rstd_all[:, 0:8], in_=mv_all[:, 0:8, 1],
                func=mybir.ActivationFunctionType.Sqrt,
                bias=eps_sb, scale=1.0,
            )
            nc.vector.reciprocal(out=rstd_all[:, 0:8], in_=rstd_all[:, 0:8])

            def rstd_rest():
                nc.scalar.activation(
                    out=rstd_all[:, 8:RPC], in_=mv_all[:, 8:RPC, 1],
                    func=mybir.ActivationFunctionType.Sqrt,
                    bias=eps_sb, scale=1.0,
                )
                nc.vector.reciprocal(
                    out=rstd_all[:, 8:RPC], in_=rstd_all[:, 8:RPC]
                )

            # pre-zero the qz (zero-blocked q) buffers; the per-pair copies
            # only touch the block-diagonal quadrants, zeros persist.
            qz_bufs = []
            for _ in range(3):
                qz = qzpool.tile([128, 2, 4, 2, S], F16, tag="qz")
                nc.gpsimd.memset(qz, 0.0)
                qz_bufs.append(qz)
            # v tiles carry an extra all-ones column per head (softmax denom);
            # pre-set it once per buffer, per-pair copies leave it intact.
            v_bufs = []
            for _ in range(3):
                v_sb = vpool.tile([S, 2, H, HD + 1], F16, tag="v")
                nc.gpsimd.memset(v_sb[:, :, :, HD:HD + 1], 1.0)
                v_bufs.append(v_sb)

            def phase1(p):
                # LN apply, transpose, QKV, full-width copies, qz, rotary
                r0 = 2 * p
                xn_pair = lnpool.tile([S, 2, D], F16, tag="xn")
                for j in range(2):
                    nc.vector.tensor_scalar(
                        out=xn_pair[:, j, :], in0=x_tiles[r0 + j],
                        scalar1=mv_all[:, r0 + j, 0:1],
                        scalar2=rstd_all[:, r0 + j:r0 + j + 1],
                        op0=mybir.AluOpType.subtract, op1=mybir.AluOpType.mult,
                    )

                # ---- transpose xn -> (d-chunk, row, tok) ----
                t_ps = ps_t.tile([128, 2, 2, S], F16, tag="tps")
                for j in range(2):
                    for c in range(2):
                        nc.tensor.transpose(
                            t_ps[:, c, j, :],
                            xn_pair[:, j, c * 128:(c + 1) * 128], ident,
                        )
                xnT_sb = tpool.tile([128, 2, 2, S], F16, tag="xnT")
                nc.scalar.copy(
                    out=xnT_sb.rearrange("p c j s -> p (c j s)"),
                    in_=t_ps.rearrange("p c j s -> p (c j s)"),
                )

                # ---- QKV projection over both rows (N=256 per mm) ----
                qk_ps = ps_qk.tile([128, 2, 2, 2, S], F32, tag="qk")  # (p,qk,ec,row,tok)
                for qk in range(2):
                    for ec in range(2):
                        for dc in range(2):
                            nc.tensor.matmul(
                                qk_ps[:, qk, ec, :, :],
                                lhsT=wqkv_sb[
                                    :, dc, qk * D + ec * 128:qk * D + (ec + 1) * 128
                                ],
                                rhs=xnT_sb[:, dc, :, :],
                                start=(dc == 0), stop=(dc == 1),
                            )
                v_ps = ps_o.tile([S, 2, D], F32, tag="ops")
                for j in range(2):
                    for dc in range(2):
                        nc.tensor.matmul(
                            v_ps[:, j, :],
                            lhsT=xnT_sb[:, dc, j, :],
                            rhs=wqkv_sb[:, dc, 2 * D:3 * D],
                            start=(dc == 0), stop=(dc == 1),
                        )

                # ---- full-width PSUM->SBUF copies (no head repack) ----
                q_sb = qkpool.tile([128, 2, 2, S], F16, tag="q")
                k_sb = qkpool.tile([128, 2, 2, S], F16, tag="k")
                nc.vector.tensor_copy(
                    out=q_sb.rearrange("p e j s -> p (e j s)"),
                    in_=qk_ps[:, 0].rearrange("p e j s -> p (e j s)"),
                )
                nc.scalar.copy(
                    out=k_sb.rearrange("p e j s -> p (e j s)"),
                    in_=qk_ps[:, 1].rearrange("p e j s -> p (e j s)"),
                )
                # v with ones column already in the buffer
                v_sb = v_bufs[p % 3]
                nc.scalar.copy(
                    out=v_sb[:, :, :, 0:HD],
                    in_=v_ps.rearrange("p j (h c) -> p j h c", c=HD),
                )

                # ---- rotary on head 0 of q and k (channels 0:32, chunk 0) ----
                cs = cos_sb[:, r0:r0 + 2, :]
                sn = sin_sb[:, r0:r0 + 2, :]
                rh_ps = ps_rh.tile([ROT, 2, 2, S], F32, tag="rh")
                nc.tensor.matmul(
                    rh_ps[:, 0], lhsT=rt_sb, rhs=q_sb[0:32, 0, :, :],
                    start=True, stop=True,
                )
                nc.tensor.matmul(
                    rh_ps[:, 1], lhsT=rt_sb, rhs=k_sb[0:32, 0, :, :],
                    start=True, stop=True,
                )
                sin_b = bass.AP(
                    tensor=sin_sb.tensor, offset=sn.offset,
                    ap=[sn.ap[0], [0, 2], sn.ap[1], sn.ap[2]],
                )
                tmp_sb = lnpool.tile([ROT, 2, 2, S], F16, tag="rtmp")
                nc.vector.tensor_mul(out=tmp_sb, in0=rh_ps, in1=sin_b)
                qz = qz_bufs[p % 3]
                # q head-0 rotary lands directly in its qz block; k in place
                nc.vector.tensor_mul(
                    out=q_sb[0:32, 0, :, :], in0=q_sb[0:32, 0, :, :], in1=cs)
                nc.vector.tensor_add(
                    out=qz[0:32, 0, 0, :, :], in0=q_sb[0:32, 0, :, :],
                    in1=tmp_sb[:, 0],
                )
                nc.gpsimd.tensor_mul(
                    out=k_sb[0:32, 0, :, :], in0=k_sb[0:32, 0, :, :], in1=cs)
                nc.gpsimd.tensor_add(
                    out=k_sb[0:32, 0, :, :], in0=k_sb[0:32, 0, :, :],
                    in1=tmp_sb[:, 1],
                )

                # ---- zero-blocked q via DMA: head g at partitions 32g,
                # block g; none of these depend on rotary ----
                for g in range(1, 4):
                    nc.sync.dma_start(
                        out=qz[32 * g:32 * (g + 1), :, g, :, :],
                        in_=q_sb[32 * g:32 * (g + 1), :, :, :],
                    )
                nc.sync.dma_start(
                    out=qz[0:32, 1, 0, :, :],
                    in_=q_sb[0:32, 1, :, :],
                )
                return {"k": k_sb, "qz": qz, "v": v_sb}

            def phase2(r, st):
                # scoresT + exp + attn@[v|1] + normalize (per row)
                k_sb, qz, v_pair = st["k"], st["qz"], st["v"]
                j = r % 2
                expT_sb = epool.tile([S, 2, 4, S], F16, tag="exp")
                for ec in range(2):
                    s_ps = ps_s.tile([S, 4, S], F32, tag="sps")
                    nc.tensor.matmul(
                        s_ps,
                        lhsT=k_sb[:, ec, j, :],
                        rhs=qz[:, ec, :, j, :],
                        start=True, stop=True,
                    )
                    nc.scalar.activation(
                        out=expT_sb[:, ec].rearrange("p h s -> p (h s)"),
                        in_=s_ps.rearrange("p h s -> p (h s)"),
                        func=mybir.ActivationFunctionType.Exp,
                        bias=maskb_sb[:, r:r + 1], scale=SCALE,
                    )

                # ---- attn @ [v | 1] ----
                o_ps = ps_o.tile([S, H, HD + 1], F32, tag="ops")
                for h in range(H):
                    nc.tensor.matmul(
                        o_ps[:, h, :],
                        lhsT=expT_sb[:, h // 4, h % 4, :],
                        rhs=v_pair[:, j, h, :],
                        start=True, stop=True,
                    )

                # ---- normalize via broadcast multiply -> (tok, h, hd) fp16 ----
                recip = apool.tile([S, H], F32, tag="recip")
                nc.vector.reciprocal(out=recip, in_=o_ps[:, :, HD])
                attn_sb = apool.tile([S, H, HD], F16, tag="attn")
                recip_b = bass.AP(
                    tensor=recip.tensor, offset=recip.offset,
                    ap=list(recip.ap) + [[0, HD]],
                )
                nc.vector.tensor_mul(
                    out=attn_sb, in0=o_ps[:, :, 0:HD], in1=recip_b
                )
                st[("attn", j)] = attn_sb

            def phase3(p, st):
                # paired: transpose attn -> (d, tok), project, store 2 rows
                r0 = 2 * p
                t2_ps = ps_t.tile([128, 2, 2, S], F16, tag="tps")
                for j in range(2):
                    attn_flat = st.pop(("attn", j)).rearrange("p h c -> p (h c)")
                    for c in range(2):
                        nc.tensor.transpose(
                            t2_ps[:, c, j, :],
                            attn_flat[:, c * 128:(c + 1) * 128], ident,
                        )
                attnT_sb = tpool.tile([128, 2, 2, S], F16, tag="attnT")
                nc.vector.tensor_copy(
                    out=attnT_sb.rearrange("p c j s -> p (c j s)"),
                    in_=t2_ps.rearrange("p c j s -> p (c j s)"),
                )

                y_ps = ps_o.tile([S, 2, D], F32, tag="ops")
                for j in range(2):
                    for c in range(2):
                        nc.tensor.matmul(
                            y_ps[:, j, :],
                            lhsT=attnT_sb[:, c, j, :],
                            rhs=wout_sb[:, c, :],
                            start=(c == 0), stop=(c == 1),
                        )
                y_sb = ypool.tile([S, 2, D], F16, tag="y")
                nc.vector.tensor_copy(
                    out=y_sb.rearrange("p j d -> p (j d)"),
                    in_=y_ps.rearrange("p j d -> p (j d)"),
                )
                nc.sync.dma_start(
                    out=y[r0:r0 + 2].rearrange("r t d -> t r d"), in_=y_sb
                )

            # software-pipelined skew over row pairs
            npairs = RPC // 2
            state = {}
            for i in range(npairs + 3):
                if i == 1:
                    rstd_rest()
                if 0 <= i - 3 < npairs:
                    phase3(i - 3, state[i - 3])
                if i < npairs:
                    state[i] = phase1(i)
                if 0 <= i - 2 < npairs:
                    for j in range(2):
                        phase2(2 * (i - 2) + j, state[i - 2])
                if 0 <= i - 3 < npairs:
                    del state[i - 3]

    nc.finalize()
    return nc


_NC = None


def _get_nc():
    global _NC
    if _NC is None:
        _NC = _build_bass()
    return _NC


def _host_prep(pair_act, pair_mask, ln_gamma, ln_beta, Wqkv, Wout):
    """Build the 8 per-core input maps (numpy only)."""
    pair_act = np.ascontiguousarray(pair_act, dtype=np.float32)
    ln_gamma = np.asarray(ln_gamma, dtype=np.float32)
    ln_beta = np.asarray(ln_beta, dtype=np.float32)
    Wqkv = np.asarray(Wqkv, dtype=np.float32)
    Wout = np.asarray(Wout, dtype=np.float32)

    W_eff = (Wqkv * ln_gamma[None, :]).T  # (256, 768): qkv = xn_z @ W_eff
    bias_eff = ln_beta @ Wqkv.T
    assert np.abs(bias_eff).max() == 0.0, "nonzero LN beta not supported"

    wqkv_h = W_eff.reshape(2, 128, 3 * D).astype(np.float16)
    wout_h = Wout.T.reshape(2, 128, D).astype(np.float16)

    # rotary tables (transposed): table[s1, c, y]
    inv_freq = 1.0 / (10000.0 ** (np.arange(0, 16, dtype=np.float32)[::2] / 16.0))
    t = np.linspace(-1.0, 1.0, S, dtype=np.float32)
    f = np.repeat(t[:, None] * inv_freq[None, :], 2, axis=-1)  # (S, 16)
    cosT = np.empty((S, ROT, S), np.float32)
    sinT = np.empty((S, ROT, S), np.float32)
    cosT[:, :16, :] = np.cos(f)[:, :, None]
    sinT[:, :16, :] = np.sin(f)[:, :, None]
    cosT[:, 16:, :] = np.cos(f).T[None, :, :]
    sinT[:, 16:, :] = np.sin(f).T[None, :, :]
    cosT = cosT.astype(np.float16)
    sinT = sinT.astype(np.float16)

    R = np.zeros((ROT, ROT), np.float32)
    for j in range(ROT // 2):
        R[2 * j, 2 * j + 1] = -1.0
        R[2 * j + 1, 2 * j] = 1.0
    rt_h = R.T.astype(np.float16)

    x_all = pair_act.reshape(NROWS, S, D)
    maskb_all = np.where(
        np.asarray(pair_mask, bool), np.float32(MASK_BIAS), np.float32(0.0)
    ).reshape(NROWS, S)

    in_maps = []
    for core in range(N_CORES):
        r0 = core * RPC
        rows = slice(r0, r0 + RPC)
        s1 = np.arange(r0, r0 + RPC) % S
        in_maps.append({
            "x": x_all[rows].astype(np.float16),
            "cos_t": np.ascontiguousarray(cosT[s1].transpose(1, 0, 2)),
            "sin_t": np.ascontiguousarray(sinT[s1].transpose(1, 0, 2)),
            "maskb": np.ascontiguousarray(maskb_all[rows].T),  # (S, RPC)
            "wqkv": wqkv_h,
            "wout": wout_h,
            "rt": rt_h,
        })
    return in_maps


def kernel(pair_act, pair_mask, ln_gamma, ln_beta, Wqkv, Wout):
    in_maps = _host_prep(pair_act, pair_mask, ln_gamma, ln_beta, Wqkv, Wout)
    nc = _get_nc()
    res = run_bass_kernel_spmd(nc, in_maps, core_ids=list(range(N_CORES)))
    y = np.stack([res.results[i]["y"] for i in range(N_CORES)])
    return y.reshape(B, S, S, D).astype(np.float32)


# revision 3
# speedup vs baseline: 1.0105x; 1.0105x over previous
"""Axial (per-row) pair attention kernel for Trainium2, 8-core SPMD.

Contract: kernel(**inputs) takes the FULL unsharded inputs from
setup_inputs() and returns the FULL (2,128,128,256) float32 output.

Sharding: the (b, s1) row axis (2*128 = 256 independent attention rows) is
split evenly across 8 NeuronCores; each core runs the identical Bass program
on its 32-row slice.

v2 changes vs baseline:
 - No per-head q/k repack. q/k are copied full-width (128 partitions) from
   PSUM, and the per-head scores matmuls are replaced by one matmul per
   (chunk, row) with lhsT = the full 128-channel k chunk and rhs = a
   zero-blocked q tile (head g's 32 channels live at partitions 32g, all
   other partitions zero), giving all 4 heads' scoresT in one N=512 matmul.
 - Softmax exp runs as one fused ACT op per (row, chunk-half) from PSUM.
 - LN-apply and the rotary cos/sin multiplies run on the idle GpSimd (Pool)
   engine (SBUF-only ops); PSUM->SBUF copies are balanced across DVE/ACT.
 - Output is DMA'd as fp16 and upcast on the host.
"""

import numpy as np

import concourse.bass as bass
import concourse.mybir as mybir
import concourse.tile as tile
from concourse import bacc
from concourse.bass_utils import run_bass_kernel_spmd
from concourse.masks import make_identity

N_CORES = 8
B, S, D = 2, 128, 256
H, HD, ROT = 8, 32, 32
NROWS = B * S
RPC = NROWS // N_CORES  # rows per core = 32
SCALE = HD ** -0.5
LN_EPS = 1e-5
MASK_BIAS = -1e9

F32 = mybir.dt.float32
F16 = mybir.dt.float16


def _build_bass() -> bass.Bass:
    nc = bacc.Bacc(None)

    x = nc.dram_tensor("x", [RPC, S, D], F16, kind="ExternalInput")
    cos_t = nc.dram_tensor("cos_t", [ROT, RPC, S], F16, kind="ExternalInput")
    sin_t = nc.dram_tensor("sin_t", [ROT, RPC, S], F16, kind="ExternalInput")
    maskb = nc.dram_tensor("maskb", [S, RPC], F32, kind="ExternalInput")
    wqkv = nc.dram_tensor("wqkv", [2, 128, 3 * D], F16, kind="ExternalInput")
    wout = nc.dram_tensor("wout", [2, 128, D], F16, kind="ExternalInput")
    rt = nc.dram_tensor("rt", [ROT, ROT], F16, kind="ExternalInput")
    y = nc.dram_tensor("y", [RPC, S, D], F16, kind="ExternalOutput")

    with tile.TileContext(nc) as tc:
        with (
            tc.tile_pool(name="consts", bufs=1) as consts,
            tc.tile_pool(name="xpool", bufs=RPC) as xpool,
            tc.tile_pool(name="lnpool", bufs=4) as lnpool,
            tc.tile_pool(name="tpool", bufs=3) as tpool,
            tc.tile_pool(name="qkpool", bufs=2) as qkpool,
            tc.tile_pool(name="qzpool", bufs=3) as qzpool,
            tc.tile_pool(name="vpool", bufs=3) as vpool,
            tc.tile_pool(name="epool", bufs=3) as epool,
            tc.tile_pool(name="apool", bufs=3) as apool,
            tc.tile_pool(name="ypool", bufs=3) as ypool,
            tc.tile_pool(name="ps_t", bufs=1, space="PSUM") as ps_t,
            tc.tile_pool(name="ps_qk", bufs=1, space="PSUM") as ps_qk,
            tc.tile_pool(name="ps_rh", bufs=1, space="PSUM") as ps_rh,
            tc.tile_pool(name="ps_s", bufs=2, space="PSUM") as ps_s,
            tc.tile_pool(name="ps_o", bufs=2, space="PSUM") as ps_o,
        ):
            # ---- constants ----
            ident = consts.tile([128, 128], F16)
            make_identity(nc, ident)
            mv_all = consts.tile([S, RPC, 2], F32)
            rstd_all = consts.tile([S, RPC], F32)
            eps_sb = consts.tile([128, 1], F32)
            nc.vector.memset(eps_sb, LN_EPS)
            x_tiles = []

            def load_pair(p):
                x_sb = xpool.tile([S, 2, D], F16)
                nc.sync.dma_start(
                    out=x_sb, in_=x[2 * p:2 * p + 2].rearrange("r t d -> t r d")
                )
                x_tiles.extend([x_sb[:, 0, :], x_sb[:, 1, :]])

            def stats_pair(p):
                stats = lnpool.tile([S, 2, 6], F32, tag="stats")
                for j in range(2):
                    nc.vector.bn_stats(
                        out=stats[:, j, :], in_=x_tiles[2 * p + j]
                    )
                    nc.vector.bn_aggr(
                        out=mv_all[:, 2 * p + j, :], in_=stats[:, j, :]
                    )

            # first 4 x pairs, then constants, then the rest of x in two
            # batched DMAs; LN stats run as the data lands
            for p in range(4):
                load_pair(p)
            wqkv_sb = consts.tile([128, 2, 3 * D], F16)
            for c in range(2):
                nc.sync.dma_start(out=wqkv_sb[:, c, :], in_=wqkv[c])
            rt_sb = consts.tile([ROT, ROT], F16)
            nc.sync.dma_start(out=rt_sb, in_=rt[:])
            maskb_sb = consts.tile([S, RPC], F32)
            nc.sync.dma_start(out=maskb_sb, in_=maskb[:])
            cos_sb = consts.tile([ROT, RPC, S], F16)
            sin_sb = consts.tile([ROT, RPC, S], F16)
            nc.sync.dma_start(out=cos_sb, in_=cos_t[:])
            nc.sync.dma_start(out=sin_sb, in_=sin_t[:])
            wout_sb = consts.tile([128, 2, D], F16)
            for c in range(2):
                nc.sync.dma_start(out=wout_sb[:, c, :], in_=wout[c])
            for p in range(4):
                stats_pair(p)
            xbig = consts.tile([S, 12, 2, D], F16)
            for h in range(2):
                nc.sync.dma_start(
                    out=xbig[:, 6 * h:6 * h + 6, :, :],
                    in_=x[8 + 12 * h:8 + 12 * h + 12].rearrange(
                        "(p r) t d -> t p r d", r=2
                    ),
                )
            for p in range(4, RPC // 2):
                x_tiles.extend(
                    [xbig[:, p - 4, 0, :], xbig[:, p - 4, 1, :]]
                )
            for p in range(4, RPC // 2):
                stats_pair(p)
            # rstd rows 0-7 now (Sqrt table loaded once); rows 8-31 at i==1,
            # still before the first Exp so the act table loads stay at two
            nc.scalar.activation(
                out=rstd_all[:, 0:8], in_=mv_all[:, 0:8, 1],
                func=mybir.ActivationFunctionType.Sqrt,
                bias=eps_sb, scale=1.0,
            )
            nc.vector.reciprocal(out=rstd_all[:, 0:8], in_=rstd_all[:, 0:8])

            def rstd_rest():
                nc.scalar.activation(
                    out=rstd_all[:, 8:RPC], in_=mv_all[:, 8:RPC, 1],
                    func=mybir.ActivationFunctionType.Sqrt,
                    bias=eps_sb, scale=1.0,
                )
                nc.vector.reciprocal(
                    out=rstd_all[:, 8:RPC], in_=rstd_all[:, 8:RPC]
                )

            # pre-zero the qz (zero-blocked q) buffers; the per-pair copies
            # only touch the block-diagonal quadrants, zeros persist.
            qz_bufs = []
            for _ in range(3):
                qz = qzpool.tile([128, 2, 4, 2, S], F16, tag="qz")
                nc.gpsimd.memset(qz, 0.0)
                qz_bufs.append(qz)
            # v tiles carry an extra all-ones column per head (softmax denom);
            # pre-set it once per buffer, per-pair copies leave it intact.
            v_bufs = []
            for _ in range(3):
                v_sb = vpool.tile([S, 2, H, HD + 1], F16, tag="v")
                nc.gpsimd.memset(v_sb[:, :, :, HD:HD + 1], 1.0)
                v_bufs.append(v_sb)

            def phase1(p):
                # LN apply, transpose, QKV, full-width copies, qz, rotary
                r0 = 2 * p
                xn_pair = lnpool.tile([S, 2, D], F16, tag="xn")
                for j in range(2):
                    nc.vector.tensor_scalar(
                        out=xn_pair[:, j, :], in0=x_tiles[r0 + j],
                        scalar1=mv_all[:, r0 + j, 0:1],
                        scalar2=rstd_all[:, r0 + j:r0 + j + 1],
                        op0=mybir.AluOpType.subtract, op1=mybir.AluOpType.mult,
                    )

                # ---- transpose xn -> (d-chunk, row, tok) ----
                t_ps = ps_t.tile([128, 2, 2, S], F16, tag="tps")
                for j in range(2):
                    for c in range(2):
                        nc.tensor.transpose(
                            t_ps[:, c, j, :],
                            xn_pair[:, j, c * 128:(c + 1) * 128], ident,
                        )
                xnT_sb = tpool.tile([128, 2, 2, S], F16, tag="xnT")
                nc.scalar.copy(
                    out=xnT_sb.rearrange("p c j s -> p (c j s)"),
                    in_=t_ps.rearrange("p c j s -> p (c j s)"),
                )

                # ---- QKV projection over both rows (N=256 per mm) ----
                qk_ps = ps_qk.tile([128, 2, 2, 2, S], F32, tag="qk")  # (p,qk,ec,row,tok)
                for qk in range(2):
                    for ec in range(2):
                        for dc in range(2):
                            nc.tensor.matmul(
                                qk_ps[:, qk, ec, :, :],
                                lhsT=wqkv_sb[
                                    :, dc, qk * D + ec * 128:qk * D + (ec + 1) * 128
                                ],
                                rhs=xnT_sb[:, dc, :, :],
                                start=(dc == 0), stop=(dc == 1),
                            )
                v_ps = ps_o.tile([S, 2, D], F32, tag="ops")
                for j in range(2):
                    for dc in range(2):
                        nc.tensor.matmul(
                            v_ps[:, j, :],
                            lhsT=xnT_sb[:, dc, j, :],
                            rhs=wqkv_sb[:, dc, 2 * D:3 * D],
                            start=(dc == 0), stop=(dc == 1),
                        )

                # ---- full-width PSUM->SBUF copies (no head repack) ----
                q_sb = qkpool.tile([128, 2, 2, S], F16, tag="q")
                k_sb = qkpool.tile([128, 2, 2, S], F16, tag="k")
                nc.vector.tensor_copy(
                    out=q_sb.rearrange("p e j s -> p (e j s)"),
                    in_=qk_ps[:, 0].rearrange("p e j s -> p (e j s)"),
                )
                nc.scalar.copy(
                    out=k_sb.rearrange("p e j s -> p (e j s)"),
                    in_=qk_ps[:, 1].rearrange("p e j s -> p (e j s)"),
                )
                # v with ones column already in the buffer
                v_sb = v_bufs[p % 3]
                nc.scalar.copy(
                    out=v_sb[:, :, :, 0:HD],
                    in_=v_ps.rearrange("p j (h c) -> p j h c", c=HD),
                )

                # ---- rotary on head 0 of q and k (channels 0:32, chunk 0) ----
                cs = cos_sb[:, r0:r0 + 2, :]
                sn = sin_sb[:, r0:r0 + 2, :]
                rh_ps = ps_rh.tile([ROT, 2, 2, S], F32, tag="rh")
                nc.tensor.matmul(
                    rh_ps[:, 0], lhsT=rt_sb, rhs=q_sb[0:32, 0, :, :],
                    start=True, stop=True,
                )
                nc.tensor.matmul(
                    rh_ps[:, 1], lhsT=rt_sb, rhs=k_sb[0:32, 0, :, :],
                    start=True, stop=True,
                )
                sin_b = bass.AP(
                    tensor=sin_sb.tensor, offset=sn.offset,
                    ap=[sn.ap[0], [0, 2], sn.ap[1], sn.ap[2]],
                )
                tmp_sb = lnpool.tile([ROT, 2, 2, S], F16, tag="rtmp")
                nc.vector.tensor_mul(out=tmp_sb, in0=rh_ps, in1=sin_b)
                qz = qz_bufs[p % 3]
                # q head-0 rotary lands directly in its qz block; k in place
                nc.vector.tensor_mul(
                    out=q_sb[0:32, 0, :, :], in0=q_sb[0:32, 0, :, :], in1=cs)
                nc.vector.tensor_add(
                    out=qz[0:32, 0, 0, :, :], in0=q_sb[0:32, 0, :, :],
                    in1=tmp_sb[:, 0],
                )
                nc.gpsimd.tensor_mul(
                    out=k_sb[0:32, 0, :, :], in0=k_sb[0:32, 0, :, :], in1=cs)
                nc.gpsimd.tensor_add(
                    out=k_sb[0:32, 0, :, :], in0=k_sb[0:32, 0, :, :],
                    in1=tmp_sb[:, 1],
                )

                # ---- zero-blocked q via DMA: head g at partitions 32g,
                # block g; none of these depend on rotary ----
                for g in range(1, 4):
                    nc.sync.dma_start(
                        out=qz[32 * g:32 * (g + 1), :, g, :, :],
                        in_=q_sb[32 * g:32 * (g + 1), :, :, :],
                    )
                nc.sync.dma_start(
                    out=qz[0:32, 1, 0, :, :],
                    in_=q_sb[0:32, 1, :, :],
                )
                return {"k": k_sb, "qz": qz, "v": v_sb}

            def phase2(r, st):
                # scoresT + exp + attn@[v|1] + normalize (per row)
                k_sb, qz, v_pair = st["k"], st["qz"], st["v"]
                j = r % 2
                expT_sb = epool.tile([S, 2, 4, S], F16, tag="exp")
                for ec in range(2):
                    s_ps = ps_s.tile([S, 4, S], F32, tag="sps")
                    nc.tensor.matmul(
                        s_ps,
                        lhsT=k_sb[:, ec, j, :],
                        rhs=qz[:, ec, :, j, :],
                        start=True, stop=True,
                    )
                    nc.scalar.activation(
                        out=expT_sb[:, ec].rearrange("p h s -> p (h s)"),
                        in_=s_ps.rearrange("p h s -> p (h s)"),
                        func=mybir.ActivationFunctionType.Exp,
                        bias=maskb_sb[:, r:r + 1], scale=SCALE,
                    )

                # ---- attn @ [v | 1] ----
                o_ps = ps_o.tile([S, H, HD + 1], F32, tag="ops")
                for h in range(H):
                    nc.tensor.matmul(
                        o_ps[:, h, :],
                        lhsT=expT_sb[:, h // 4, h % 4, :],
                        rhs=v_pair[:, j, h, :],
                        start=True, stop=True,
                    )

                # ---- normalize via broadcast multiply -> (tok, h, hd) fp16 ----
                recip = apool.tile([S, H], F32, tag="recip")
                nc.vector.reciprocal(out=recip, in_=o_ps[:, :, HD])
                attn_sb = apool.tile([S, H, HD], F16, tag="attn")
                recip_b = bass.AP(
                    tensor=recip.tensor, offset=recip.offset,
                    ap=list(recip.ap) + [[0, HD]],
                )
                nc.vector.tensor_mul(
                    out=attn_sb, in0=o_ps[:, :, 0:HD], in1=recip_b
                )
                st[("attn", j)] = attn_sb

            def phase3(p, st):
                # paired: transpose attn -> (d, tok), project, store 2 rows
                r0 = 2 * p
                t2_ps = ps_t.tile([128, 2, 2, S], F16, tag="tps")
                for j in range(2):
                    attn_flat = st.pop(("attn", j)).rearrange("p h c -> p (h c)")
                    for c in range(2):
                        nc.tensor.transpose(
                            t2_ps[:, c, j, :],
                            attn_flat[:, c * 128:(c + 1) * 128], ident,
                        )
                attnT_sb = tpool.tile([128, 2, 2, S], F16, tag="attnT")
                nc.vector.tensor_copy(
                    out=attnT_sb.rearrange("p c j s -> p (c j s)"),
                    in_=t2_ps.rearrange("p c j s -> p (c j s)"),
                )

                y_ps = ps_o.tile([S, 2, D], F32, tag="ops")
                for j in range(2):
                    for c in range(2):
                        nc.tensor.matmul(
                            y_ps[:, j, :],
                            lhsT=attnT_sb[:, c, j, :],
                            rhs=wout_sb[:, c, :],
                            start=(c == 0), stop=(c == 1),
                        )
                y_sb = ypool.tile([S, 2, D], F16, tag="y")
                nc.vector.tensor_copy(
                    out=y_sb.rearrange("p j d -> p (j d)"),
                    in_=y_ps.rearrange("p j d -> p (j d)"),
                )
                nc.scalar.dma_start(
                    out=y[r0:r0 + 2].rearrange("r t d -> t r d"), in_=y_sb
                )

            # software-pipelined skew over row pairs
            npairs = RPC // 2
            state = {}
            for i in range(npairs + 3):
                if i == 1:
                    rstd_rest()
                if 0 <= i - 3 < npairs:
                    phase3(i - 3, state[i - 3])
                if i < npairs:
                    state[i] = phase1(i)
                if 0 <= i - 2 < npairs:
                    for j in range(2):
                        phase2(2 * (i - 2) + j, state[i - 2])
                if 0 <= i - 3 < npairs:
                    del state[i - 3]

    nc.finalize()
    return nc


_NC = None


def _get_nc():
    global _NC
    if _NC is None:
        _NC = _build_bass()
    return _NC


def _host_prep(pair_act, pair_mask, ln_gamma, ln_beta, Wqkv, Wout):
    """Build the 8 per-core input maps (numpy only)."""
    pair_act = np.ascontiguousarray(pair_act, dtype=np.float32)
    ln_gamma = np.asarray(ln_gamma, dtype=np.float32)
    ln_beta = np.asarray(ln_beta, dtype=np.float32)
    Wqkv = np.asarray(Wqkv, dtype=np.float32)
    Wout = np.asarray(Wout, dtype=np.float32)

    W_eff = (Wqkv * ln_gamma[None, :]).T  # (256, 768): qkv = xn_z @ W_eff
    bias_eff = ln_beta @ Wqkv.T
    assert np.abs(bias_eff).max() == 0.0, "nonzero LN beta not supported"

    wqkv_h = W_eff.reshape(2, 128, 3 * D).astype(np.float16)
    wout_h = Wout.T.reshape(2, 128, D).astype(np.float16)

    # rotary tables (transposed): table[s1, c, y]
    inv_freq = 1.0 / (10000.0 ** (np.arange(0, 16, dtype=np.float32)[::2] / 16.0))
    t = np.linspace(-1.0, 1.0, S, dtype=np.float32)
    f = np.repeat(t[:, None] * inv_freq[None, :], 2, axis=-1)  # (S, 16)
    cosT = np.empty((S, ROT, S), np.float32)
    sinT = np.empty((S, ROT, S), np.float32)
    cosT[:, :16, :] = np.cos(f)[:, :, None]
    sinT[:, :16, :] = np.sin(f)[:, :, None]
    cosT[:, 16:, :] = np.cos(f).T[None, :, :]
    sinT[:, 16:, :] = np.sin(f).T[None, :, :]
    cosT = cosT.astype(np.float16)
    sinT = sinT.astype(np.float16)

    R = np.zeros((ROT, ROT), np.float32)
    for j in range(ROT // 2):
        R[2 * j, 2 * j + 1] = -1.0
        R[2 * j + 1, 2 * j] = 1.0
    rt_h = R.T.astype(np.float16)

    x_all = pair_act.reshape(NROWS, S, D)
    maskb_all = np.where(
        np.asarray(pair_mask, bool), np.float32(MASK_BIAS), np.float32(0.0)
    ).reshape(NROWS, S)

    in_maps = []
    for core in range(N_CORES):
        r0 = core * RPC
        rows = slice(r0, r0 + RPC)
        s1 = np.arange(r0, r0 + RPC) % S
        in_maps.append({
            "x": x_all[rows].astype(np.float16),
            "cos_t": np.ascontiguousarray(cosT[s1].transpose(1, 0, 2)),
            "sin_t": np.ascontiguousarray(sinT[s1].transpose(1, 0, 2)),
            "maskb": np.ascontiguousarray(maskb_all[rows].T),  # (S, RPC)
            "wqkv": wqkv_h,
            "wout": wout_h,
            "rt": rt_h,
        })
    return in_maps


def kernel(pair_act, pair_mask, ln_gamma, ln_beta, Wqkv, Wout):
    in_maps = _host_prep(pair_act, pair_mask, ln_gamma, ln_beta, Wqkv, Wout)
    nc = _get_nc()
    res = run_bass_kernel_spmd(nc, in_maps, core_ids=list(range(N_CORES)))
    y = np.stack([res.results[i]["y"] for i in range(N_CORES)])
    return y.reshape(B, S, S, D).astype(np.float32)
